# revision 25
# baseline (speedup 1.0000x reference)
"""TRN2 Bass kernel for nn_Attn_50233937494279.

Cross-attention block, data-parallel over batch (16) across 8 NeuronCores
(2 batches/core). Per batch, on device:

  qb     = q @ W_in.T                  (GEMM1, 3-pass fp16 hi/lo split)
  logits = qb @ c.T                    (GEMM2, 3-pass fp16 hi/lo split)
  score  = softmax(logits, axis=-1)    (online softmax, fp32 stats, fp16 p)
  ctx    = score @ c                   (GEMM3, 1-pass fp16)
  out    = tanh([ctx, qb] @ W_out.T)   (GEMM4, 1-pass fp16)

Layout/engine strategy:
- All transposes use the DMA xbar (fp16, SBUF->SBUF) on the nc.sync HWDGE
  queue; fp32 staging loads ride nc.scalar's HWDGE queue; c and W_out fp16
  "hi" parts are cast-loaded straight from DRAM via SWDGE (nc.gpsimd),
  which also absorbs the c-split subs and the score-normalize multiplies so
  DVE/ACT stay on the softmax/psum-drain critical path only.
- Unnormalized exp(logits - running_max) stays in SBUF as fp16; the final
  per-row rescale folds the running-max correction and the softmax
  denominator into one scalar multiply.
- The host only re-lays-out data: batch-major sharding of q/c, and
  W_in/W_out passed pre-transposed (pure relayout of replicated weights;
  all arithmetic happens on device).
"""

from contextlib import ExitStack

import numpy as np

import concourse.bacc as bacc
import concourse.tile as tile
from concourse import mybir
from concourse.bass_utils import run_bass_kernel_spmd

F32 = mybir.dt.float32
F16 = mybir.dt.float16  # 11-bit mantissa at the same PE cost as bf16
AX = mybir.AxisListType.X
EXP = mybir.ActivationFunctionType.Exp
TANH = mybir.ActivationFunctionType.Tanh

DIM = 1024      # feature dim
QL = 1024       # q_len
CL = 2048       # c_len
BPC = 2         # batches per core
NCORES = 8
NQT = QL // 128     # 8 q row-tiles
NDT = DIM // 128    # 8 feature tiles
NKT = CL // 128     # 16 k row-tiles
KC = 512            # k chunk width
NCH = CL // KC      # 4 k chunks
QC = 512            # matmul moving-operand free width


def build_nc():
    nc = bacc.Bacc("TRN2", target_bir_lowering=False, debug=False,
                   num_devices=NCORES)

    qin = nc.dram_tensor("qin", [BPC, QL, DIM], F32, kind="ExternalInput").ap()
    cin = nc.dram_tensor("cin", [BPC, CL, DIM], F32, kind="ExternalInput").ap()
    w_inT = nc.dram_tensor("w_inT", [DIM, DIM], F32, kind="ExternalInput").ap()
    w_outT = nc.dram_tensor("w_outT", [2 * DIM, DIM], F32, kind="ExternalInput").ap()
    out = nc.dram_tensor("out", [BPC, QL, DIM], F32, kind="ExternalOutput").ap()
    score = nc.dram_tensor("score", [BPC, QL, CL], F32, kind="ExternalOutput").ap()

    st = {}  # cross-phase state, keyed per batch

    with tile.TileContext(nc) as tc, ExitStack() as ctx:
        big = ctx.enter_context(tc.tile_pool(name="big", bufs=1))
        wo_bf = []

        def prep_wout():
            for ft in range(2 * NDT):
                wb = big.tile([128, DIM], F16, tag=f"wo_bf{ft}",
                              name=f"wo_bf{ft}")
                nc.gpsimd.dma_start(wb[:], w_outT[ft * 128:(ft + 1) * 128, :])
                wo_bf.append(wb)

        def phase_ab(bi):
            """q^T + GEMM1 -> qbT hi/lo."""
            qbT_hi = [big.tile([128, QL], F16, tag=f"qbT_hi{i}",
                               name=f"qbT_hi{i}") for i in range(NDT)]
            qbT_lo = [big.tile([128, QL], F16, tag=f"qbT_lo{i}",
                               name=f"qbT_lo{i}") for i in range(NDT)]
            st[bi] = {"qbT_hi": qbT_hi, "qbT_lo": qbT_lo}
            with tc.tile_pool(name="ab", bufs=1) as pab, \
                 tc.tile_pool(name="ab_w", bufs=3) as pwl, \
                 tc.tile_pool(name="ab_st", bufs=2) as pst, \
                 tc.tile_pool(name="ps_ab", bufs=8, space="PSUM") as psA:
                # qT holds hi (m=0..7) and lo (m=8..15) d-tiles
                qT = pab.tile([128, 2 * NDT, QL], F16, tag="qT", name="qT")
                for qt in range(NQT):
                    qsl = slice(qt * 128, (qt + 1) * 128)
                    qf = pst.tile([128, DIM], F32, tag="q_stage", name="q_stage")
                    nc.scalar.dma_start(qf[:], qin[bi, qsl, :])
                    qcat = pst.tile([128, 2 * DIM], F16, tag="qcat", name="qcat",
                                    bufs=3)
                    nc.vector.tensor_copy(qcat[:, 0:DIM], qf[:])
                    nc.vector.tensor_sub(qcat[:, DIM:2 * DIM], qf[:],
                                         qcat[:, 0:DIM])
                    nc.sync.dma_start(qT[:, :, qsl], qcat[:], transpose=True)

                for qc in range(QL // QC):
                    pgs = [psA.tile([128, QC], F32, tag="pg1", name="pg1")
                           for _ in range(NDT)]
                    for dt in range(NDT):
                        dsl = slice(dt * 128, (dt + 1) * 128)
                        wf = pst.tile([128, DIM], F32, tag="w_stage", name="w_stage")
                        nc.scalar.dma_start(wf[:], w_inT[dsl, :])
                        whi = pwl.tile([128, DIM], F16, tag="w_hi", name="w_hi")
                        nc.vector.tensor_copy(whi[:], wf[:])
                        wlo = pwl.tile([128, DIM], F16, tag="w_lo", name="w_lo")
                        nc.vector.tensor_sub(wlo[:], wf[:], whi[:])
                        for et in range(NDT):
                            esl = slice(et * 128, (et + 1) * 128)
                            for p, (L, m) in enumerate(
                                    ((whi, dt), (whi, NDT + dt), (wlo, dt))):
                                nc.tensor.matmul(
                                    pgs[et][:],
                                    L[:, esl],
                                    qT[:, m, qc * QC:(qc + 1) * QC],
                                    start=(dt == 0 and p == 0),
                                    stop=(dt == NDT - 1 and p == 2))
                    sl = slice(qc * QC, (qc + 1) * QC)
                    for et in range(NDT):
                        nc.vector.tensor_copy(qbT_hi[et][:, sl], pgs[et][:])
                        nc.vector.tensor_sub(qbT_lo[et][:, sl], pgs[et][:],
                                             qbT_hi[et][:, sl])

        def phase_c(bi, pp):
            """Stream c; GEMM2 3-pass; online softmax -> fp16 p in SBUF."""
            qbT_hi, qbT_lo = st[bi]["qbT_hi"], st[bi]["qbT_lo"]
            chi = [big.tile([128, DIM], F16, tag=f"chi{i}", name=f"chi{i}")
                   for i in range(NKT)]
            nm_run = big.tile([128, NQT], F32, tag="nm_run", name="nm_run")
            s_run = big.tile([128, NQT], F32, tag="s_run", name="s_run")
            nmu = big.tile([128, NQT, NCH], F32, tag="nmu", name="nmu")
            p_sb = [pp.tile([128, CL], F16, tag=f"p{i}", name=f"p{i}")
                    for i in range(NQT)]
            st[bi].update(chi=chi, nm_run=nm_run, s_run=s_run, nmu=nmu, p_sb=p_sb)
            with tc.tile_pool(name="c_st", bufs=2) as pcst, \
                 tc.tile_pool(name="c_sm", bufs=4) as psm, \
                 tc.tile_pool(name="ps_c", bufs=8, space="PSUM") as psC:
                for chn in range(NCH):
                    # cT holds hi (m=0..7) and lo (m=8..15) feature tiles
                    cT = pcst.tile([128, 2 * NDT, KC], F16, tag="cT", name="cT")
                    for ct in range(KC // 128):
                        kt = chn * (KC // 128) + ct
                        ksl = slice(kt * 128, (kt + 1) * 128)
                        cf = pcst.tile([128, DIM], F32, tag="c_stage",
                                       name="c_stage", bufs=3)
                        nc.scalar.dma_start(cf[:], cin[bi, ksl, :])
                        nc.gpsimd.dma_start(chi[kt][:], cin[bi, ksl, :])
                        ccat = pcst.tile([128, 2 * DIM], F16, tag="ccat",
                                         name="ccat", bufs=3)
                        nc.gpsimd.tensor_copy(ccat[:, 0:DIM], chi[kt][:])
                        nc.gpsimd.tensor_sub(ccat[:, DIM:2 * DIM], cf[:],
                                             chi[kt][:])
                        tsl = slice(ct * 128, (ct + 1) * 128)
                        nc.sync.dma_start(cT[:, :, tsl], ccat[:], transpose=True)

                    nrm = psm.tile([128, NQT], F32, tag="nrm", name="nrm")
                    ps = psm.tile([128, NQT], F32, tag="ps", name="ps")
                    alpha = (psm.tile([128, NQT], F32, tag="alpha", name="alpha")
                             if chn > 0 else None)
                    for g in range(2):          # mt groups of 4
                        gs = slice(4 * g, 4 * g + 4)
                        pgs = []
                        for mt in range(4 * g, 4 * g + 4):
                            pg = psC.tile([128, KC], F32, tag="pg2", name="pg2")
                            n = 0
                            for et in range(NDT):
                                msl = slice(mt * 128, (mt + 1) * 128)
                                for L, m in ((qbT_hi[et], et),
                                             (qbT_hi[et], NDT + et),
                                             (qbT_lo[et], et)):
                                    nc.tensor.matmul(
                                        pg[:], L[:, msl], cT[:, m, :],
                                        start=(n == 0), stop=(n == 23))
                                    n += 1
                            nc.vector.reduce_max(nrm[:, mt:mt + 1], pg[:],
                                                 axis=AX, negate=True)
                            pgs.append(pg)
                        if chn == 0:
                            nc.vector.tensor_copy(nmu[:, gs, chn], nrm[:, gs])
                            nc.vector.tensor_copy(nm_run[:, gs], nrm[:, gs])
                        else:
                            nc.vector.tensor_tensor(nmu[:, gs, chn],
                                                    nm_run[:, gs], nrm[:, gs],
                                                    op=mybir.AluOpType.min)
                            # alpha = exp(m_old - m_new); then commit new max
                            for mt in range(4 * g, 4 * g + 4):
                                nc.scalar.activation(
                                    alpha[:, mt:mt + 1], nm_run[:, mt:mt + 1],
                                    EXP, scale=-1.0, bias=nmu[:, mt, chn:chn + 1])
                            nc.vector.tensor_copy(nm_run[:, gs], nmu[:, gs, chn])
                        for i, mt in enumerate(range(4 * g, 4 * g + 4)):
                            nc.scalar.activation(
                                p_sb[mt][:, chn * KC:(chn + 1) * KC], pgs[i][:],
                                EXP, bias=nmu[:, mt, chn:chn + 1],
                                accum_out=ps[:, mt:mt + 1])
                        if chn == 0:
                            nc.vector.tensor_copy(s_run[:, gs], ps[:, gs])
                        else:
                            nc.vector.tensor_mul(s_run[:, gs], s_run[:, gs],
                                                 alpha[:, gs])
                            nc.vector.tensor_add(s_run[:, gs], s_run[:, gs],
                                                 ps[:, gs])

        def phase_d_pt(bi, pp, pdt):
            """Rescale p -> pT (fp16) for GEMM3. Gates GEMM3; keep it short."""
            nm_run, s_run, nmu = st[bi]["nm_run"], st[bi]["s_run"], st[bi]["nmu"]
            p_sb = st[bi]["p_sb"]
            pT = pdt.tile([128, NKT, QL], F16, tag="pT", name="pT")
            st[bi]["pT"] = pT
            rcp = pp.tile([128, NQT], F32, tag="rcp", name="rcp")
            nc.vector.reciprocal(rcp[:], s_run[:])
            fac = pp.tile([128, NQT, NCH], F32, tag="fac", name="fac")
            st[bi]["fac"] = fac
            for mt in range(NQT):
                nc.scalar.activation(fac[:, mt, :], nmu[:, mt, :], EXP,
                                     scale=-1.0, bias=nm_run[:, mt:mt + 1])
                nc.vector.tensor_scalar_mul(fac[:, mt, :], fac[:, mt, :],
                                            rcp[:, mt:mt + 1])
            with tc.tile_pool(name="d_st", bufs=3) as pdst:
                for mt in range(NQT):
                    pn = pdst.tile([128, CL], F16, tag="pn", name="pn")
                    for chn in range(NCH):
                        nc.vector.tensor_scalar_mul(
                            pn[:, chn * KC:(chn + 1) * KC],
                            p_sb[mt][:, chn * KC:(chn + 1) * KC],
                            fac[:, mt, chn:chn + 1])
                    nc.sync.dma_start(
                        pT[:, :, mt * 128:(mt + 1) * 128], pn[:],
                        transpose=True)

        def phase_d_score(bi):
            """Normalized score output, entirely on POOL + SWDGE."""
            p_sb, fac = st[bi]["p_sb"], st[bi]["fac"]
            with tc.tile_pool(name="dsc_st", bufs=3) as pdsc:
                for mt in range(NQT):
                    sn = pdsc.tile([128, CL], F32, tag="sn", name="sn")
                    for chn in range(NCH):
                        nc.gpsimd.tensor_scalar_mul(
                            sn[:, chn * KC:(chn + 1) * KC],
                            p_sb[mt][:, chn * KC:(chn + 1) * KC],
                            fac[:, mt, chn:chn + 1])
                    nc.gpsimd.dma_start(score[bi, mt * 128:(mt + 1) * 128, :],
                                        sn[:])

        def phase_e(bi):
            """GEMM3: ctx^T = c^T @ p^T."""
            chi, pT = st[bi]["chi"], st[bi]["pT"]
            ctxT = [big.tile([128, QL], F16, tag=f"ctxT{i}", name=f"ctxT{i}")
                    for i in range(NDT)]
            st[bi]["ctxT"] = ctxT
            with tc.tile_pool(name="ps_e", bufs=4, space="PSUM") as psE:
                for dt in range(NDT):
                    for qc in range(QL // QC):
                        pg = psE.tile([128, QC], F32, tag="pg3", name="pg3")
                        for kt in range(NKT):
                            nc.tensor.matmul(
                                pg[:],
                                chi[kt][:, dt * 128:(dt + 1) * 128],
                                pT[:, kt, qc * QC:(qc + 1) * QC],
                                start=(kt == 0), stop=(kt == NKT - 1))
                        nc.vector.tensor_copy(ctxT[dt][:, qc * QC:(qc + 1) * QC],
                                              pg[:])

        def phase_f(bi):
            """GEMM4 + tanh -> out."""
            qbT_hi, ctxT = st[bi]["qbT_hi"], st[bi]["ctxT"]
            with tc.tile_pool(name="f_st", bufs=2) as pfst, \
                 tc.tile_pool(name="ps_f", bufs=4, space="PSUM") as psF:
                for mt in range(NQT):
                    ob = pfst.tile([128, DIM], F32, tag="ob", name="ob")
                    for dc in range(DIM // QC):
                        pg = psF.tile([128, QC], F32, tag="pg4", name="pg4")
                        for ft in range(2 * NDT):
                            L = ctxT[ft] if ft < NDT else qbT_hi[ft - NDT]
                            nc.tensor.matmul(
                                pg[:],
                                L[:, mt * 128:(mt + 1) * 128],
                                wo_bf[ft][:, dc * QC:(dc + 1) * QC],
                                start=(ft == 0), stop=(ft == 2 * NDT - 1))
                        nc.scalar.activation(ob[:, dc * QC:(dc + 1) * QC], pg[:],
                                             TANH)
                    nc.scalar.dma_start(out[bi, mt * 128:(mt + 1) * 128, :], ob[:])

        for bi in range(BPC):
            phase_ab(bi)
            with tc.tile_pool(name="pp", bufs=1) as pp:
                phase_c(bi, pp)
                with tc.tile_pool(name="pdt", bufs=1) as pdt:
                    phase_d_pt(bi, pp, pdt)
                    if bi == 0:
                        prep_wout()
                    phase_d_score(bi)
                    phase_e(bi)
                phase_f(bi)

    nc.compile()
    return nc


_NC_CACHE = None


def _get_nc():
    global _NC_CACHE
    if _NC_CACHE is None:
        _NC_CACHE = build_nc()
    return _NC_CACHE


def run(q, c, W_in, W_out, trace=False, **spmd_kwargs):
    nc = _get_nc()
    w_inT = np.ascontiguousarray(W_in.T).astype(np.float32, copy=False)
    w_outT = np.ascontiguousarray(W_out.T).astype(np.float32, copy=False)
    in_maps = []
    for core in range(NCORES):
        bsl = slice(BPC * core, BPC * (core + 1))
        in_maps.append({
            "qin": np.ascontiguousarray(np.transpose(q[:, bsl, :], (1, 0, 2))),
            "cin": np.ascontiguousarray(np.transpose(c[:, bsl, :], (1, 0, 2))),
            "w_inT": w_inT,
            "w_outT": w_outT,
        })
    res = run_bass_kernel_spmd(nc, in_maps, core_ids=list(range(NCORES)),
                               trace=trace, **spmd_kwargs)
    outs = np.concatenate([r["out"] for r in res.results], axis=0)
    scores = np.concatenate([r["score"] for r in res.results], axis=0)
    out_full = np.ascontiguousarray(np.transpose(outs, (1, 0, 2)))
    return (out_full, scores), res


def kernel(q, c, W_in, W_out):
    (out_full, scores), _ = run(q, c, W_in, W_out)
    return out_full, scores
